# revision 9
# baseline (speedup 1.0000x reference)
"""Trainium2 Bass kernel for a dense transformer block (B=8, T=1024, C=1024, H=16).

Data-parallel over batch across the 8 NeuronCores (one batch element per core,
weights replicated, no collectives).

Per-core dataflow v2 (fp8 DoubleRow for qkv/proj, bf16 MLP, packed scores):

  per ti: x chunk DMA -> LN1 -> PE transpose -> h1T fp8 [C,T]
          -> V(ti) = h @ Wv   (fp8 DoubleRow)        -> vaug bf16 [T,H,D+1]
  per pair c (heads 2c, 2c+1):
    qkT chunks (fp8 DoubleRow, lhsT=Wqk)             -> qk_c bf16 [128, 2, T]
    scores S^T: both heads packed on PE row halves via tile_position
    E = exp(S/8) on ACT (bf16), causal diag mask post-exp (DVE)
    yT_aug halves = E^T-blocks @ [v|1]  (bf16)       -> psum [65, 512] x2
    den recip on psum row 64 (DVE) -> partition_broadcast (GPSIMD)
    yT = psum * recip -> fp8, DMA into yT8 (handles odd-head partition shift)
  proj = y @ Wp (fp8 DoubleRow) + x                  -> r1 (natural f32)
  LN2 -> h2T bf16; fc1+gelu (bf16) -> aT; fc2 + residual (bf16) -> out

All PSUM accumulation fp32; LN statistics and residual stream fp32.
"""
import sys

sys.path.insert(0, "/opt/trn_rl_repo")

import numpy as np
import ml_dtypes

import concourse.bass as bass
import concourse.tile as tile
from concourse import mybir
from concourse.masks import make_identity
from concourse.vector_clock import ScopedClock

F32 = mybir.dt.float32
BF16 = mybir.dt.bfloat16
FP8 = mybir.dt.float8e4
AF = mybir.ActivationFunctionType
DR = mybir.MatmulPerfMode.DoubleRow

T, C, H, D = 1024, 1024, 16, 64
NT = T // 128   # 8 token chunks
NC_ = C // 128  # 8 feature chunks
EPS = 1e-5

# ---------------------------------------------------------------------------
# Walrus in this container rejects >1 sem-wait per CTRL instruction; split the
# Tile tail-drain's waits across nop carriers. The carrier nops (wait-only, no
# updates) break CoreSim's race detector, so sim builds disable this shim.
_MAX_WAITS = 1
_WALRUS_COMPAT = True
_SIM_GELU_TANH = False

_orig_drain_and_barrier = tile.TileContext._drain_and_barrier


def _patched_drain_and_barrier(self, tick_clock, wait_clock):
    if not _WALRUS_COMPAT:
        return _orig_drain_and_barrier(self, tick_clock, wait_clock)
    nc = self.nc
    carrier = nc.sync.nop(nofuse=True)
    wait_clock.add_sem_waits(carrier.ins, ScopedClock({None: tick_clock.global_clock}))
    si = carrier.ins.sync_info
    waits = list(si.on_wait) if si and si.on_wait else []
    if len(waits) > _MAX_WAITS:
        si.on_wait = waits[:_MAX_WAITS]
        for k in range(_MAX_WAITS, len(waits), _MAX_WAITS):
            extra = nc.sync.nop(nofuse=True)
            esi = extra.ins.sync_info
            if esi is None:
                extra.ins.sync_info = mybir.SyncInfo(
                    on_wait=waits[k:k + _MAX_WAITS], on_update=[]
                )
            else:
                esi.on_wait = waits[k:k + _MAX_WAITS]
    nc.sync.drain()
    nc.all_engine_barrier()
    popped = nc._tile_sem_poison_stack.pop()
    assert popped is self._sem_poison
    nc.clear_and_free_semaphores(list(self.sems.allocated().values()))
    nc.all_engine_barrier()


tile.TileContext._drain_and_barrier = _patched_drain_and_barrier


def _split_sync_waits(nc, max_waits=1):
    """Walrus here rejects >1 sem-wait per instruction; hoist extras onto
    preceding same-engine nops."""
    ctr = 0
    for f in nc.m.functions:
        for b in f.blocks:
            out = []
            for ins in b.instructions:
                si = ins.sync_info
                ws = list(si.on_wait) if si and si.on_wait else []
                if len(ws) > max_waits:
                    extra, keep = ws[:-max_waits], ws[-max_waits:]
                    for i in range(0, len(extra), max_waits):
                        nop = mybir.InstNoOp(
                            name=f"wsplit-{ctr}", engine=ins.engine,
                            sync_info=mybir.SyncInfo(
                                on_wait=extra[i:i + max_waits], on_update=[]))
                        ctr += 1
                        out.append(nop)
                    si.on_wait = keep
                out.append(ins)
            b.instructions = out


def build_nc(flags):
    nc = bass.Bass()

    x_d = nc.dram_tensor("x", [T, C], F32, kind="ExternalInput")
    # host-prearranged: [m_chunk, p, ko, 128] so per-chunk DMAs are contiguous
    wqk_d = nc.dram_tensor("w_qk", [2 * NC_, 128, NC_, 128], FP8,
                           kind="ExternalInput")
    wfc_d = nc.dram_tensor("w_fc", [4 * NC_, 128, NC_, 128], BF16,
                           kind="ExternalInput")
    wv_d = nc.dram_tensor("w_v", [C, C], FP8, kind="ExternalInput")
    wp_d = nc.dram_tensor("w_proj", [C, C], FP8, kind="ExternalInput")
    wmlp_d = nc.dram_tensor("w_mlp", [4 * C, C], BF16, kind="ExternalInput")
    mask_d = nc.dram_tensor("mask_ut", [128, 128], BF16, kind="ExternalInput")
    opt = {}
    if flags["b_qk"]:
        opt["b_qk"] = nc.dram_tensor("b_qk", [128, 2 * NC_], F32, kind="ExternalInput")
    if flags["b_v"]:
        opt["b_v"] = nc.dram_tensor("b_v", [C], F32, kind="ExternalInput")
    if flags["b_proj"]:
        opt["b_proj"] = nc.dram_tensor("b_proj", [C], F32, kind="ExternalInput")
    if flags["b_fc"]:
        opt["b_fc"] = nc.dram_tensor("b_fc", [128, 4 * NC_], F32, kind="ExternalInput")
    if flags["b_mlp"]:
        opt["b_mlp"] = nc.dram_tensor("b_mlp", [C], F32, kind="ExternalInput")
    for nm in ("ln1_g", "ln1_b", "ln2_g", "ln2_b"):
        if flags[nm]:
            opt[nm] = nc.dram_tensor(nm, [C], F32, kind="ExternalInput")
    out_d = nc.dram_tensor("out", [T, C], F32, kind="ExternalOutput")

    with tile.TileContext(nc) as tc:
        _build_body(nc, tc, flags, x_d, wqk_d, wv_d, wp_d, wfc_d, wmlp_d,
                    mask_d, opt, out_d)
    if _WALRUS_COMPAT:
        _split_sync_waits(nc)
    return nc


def _build_body(nc, tc, flags, x_d, wqk_d, wv_d, wp_d, wfc_d, wmlp_d,
                mask_d, opt, out_d):
    from contextlib import ExitStack

    ctx = ExitStack()
    with ctx:
        const = ctx.enter_context(tc.tile_pool(name="const", bufs=1))
        big = ctx.enter_context(tc.tile_pool(name="big", bufs=1))
        scratch = ctx.enter_context(tc.tile_pool(name="scratch", bufs=3))
        small = ctx.enter_context(tc.tile_pool(name="small", bufs=8))
        o_pool = ctx.enter_context(tc.tile_pool(name="opool", bufs=2))
        dram = ctx.enter_context(tc.tile_pool(name="dram", bufs=1, space="DRAM"))

        # ---- constants -----------------------------------------------------
        ident = const.tile([128, 128], BF16, tag="ident")
        make_identity(nc, ident)
        mask_sb = const.tile([128, 128], BF16, tag="mask")
        nc.sync.dma_start(mask_sb[:], mask_d[:])
        eps_t = const.tile([128, 1], F32, tag="eps")
        nc.vector.memset(eps_t[:], EPS)

        def rep128(vec_dram):
            t = const.tile([128, C], F32, tag=f"rep_{vec_dram.tensor.name}")
            src = bass.AP(tensor=vec_dram.tensor, offset=0, ap=[[0, 128], [1, C]])
            nc.gpsimd.dma_start(out=t[:], in_=src)
            return t

        ln1_g_rep = rep128(opt["ln1_g"]) if flags["ln1_g"] else None
        ln1_b_rep = rep128(opt["ln1_b"]) if flags["ln1_b"] else None
        ln2_g_rep = rep128(opt["ln2_g"]) if flags["ln2_g"] else None
        ln2_b_rep = rep128(opt["ln2_b"]) if flags["ln2_b"] else None
        bv_rep = rep128(opt["b_v"]) if flags["b_v"] else None
        bproj_rep = rep128(opt["b_proj"]) if flags["b_proj"] else None
        bmlp_rep = rep128(opt["b_mlp"]) if flags["b_mlp"] else None
        bqk_sb = None
        if flags["b_qk"]:
            bqk_sb = const.tile([128, 2 * NC_], F32, tag="bqk")
            nc.sync.dma_start(bqk_sb[:], opt["b_qk"][:])
        bfc_sb = None
        if flags["b_fc"]:
            bfc_sb = const.tile([128, 4 * NC_], F32, tag="bfc")
            nc.sync.dma_start(bfc_sb[:], opt["b_fc"][:])

        # ---- persistent tiles ---------------------------------------------
        x_sb = big.tile([128, NT, C], F32, tag="x")        # x, then r1 in place
        bufT8 = big.tile([128, NC_, T], FP8, tag="bufT8")  # h1T fp8
        bufTb = big.tile([128, NC_, T], BF16, tag="bufTb")  # h2T bf16
        yT8 = big.tile([128, NC_, T], FP8, tag="yT8")      # normalized y^T fp8

        def layernorm_chunk(src_slice, g_rep, b_rep):
            stats = small.tile([128, 2, 6], F32, tag="bn_stats")
            xr = src_slice.rearrange("p (s f) -> p s f", f=512)
            for s in range(2):
                nc.vector.bn_stats(out=stats[:, s, :], in_=xr[:, s, :])
            mv = small.tile([128, 2], F32, tag="bn_mv")
            nc.vector.bn_aggr(out=mv[:], in_=stats[:])
            rstd = small.tile([128, 1], F32, tag="rstd")
            nc.scalar.activation(out=rstd[:], in_=mv[:, 1:2], func=AF.Sqrt,
                                 bias=eps_t[:], scale=1.0)
            nc.vector.reciprocal(out=rstd[:], in_=rstd[:])
            h_blk = scratch.tile([128, C], BF16, tag="h_blk")
            nc.vector.tensor_scalar(
                out=h_blk[:], in0=src_slice, scalar1=mv[:, 0:1], scalar2=rstd[:],
                op0=mybir.AluOpType.subtract, op1=mybir.AluOpType.mult)
            if g_rep is not None:
                nc.vector.tensor_mul(h_blk[:], h_blk[:], g_rep[:])
            if b_rep is not None:
                nc.vector.tensor_add(h_blk[:], h_blk[:], b_rep[:])
            return h_blk

        # PSUM pools: ps_big = two 2-bank slots ([128, 1024] f32 accumulators
        # for qk/V/proj/fc1); ps_att = four 1-bank slots (transposes, score
        # tiles, PV halves, warmup). 2*2 + 4*1 = 8 banks.
        ps_ctx = ExitStack()
        ps_big = ps_ctx.enter_context(
            tc.tile_pool(name="ps_big", bufs=2, space="PSUM"))
        ps_att = ps_ctx.enter_context(
            tc.tile_pool(name="ps_att", bufs=4, space="PSUM"))

        def transpose_into(dst, dst_ti, src_blk):
            for jc in range(NC_):
                pst = ps_att.tile([128, 128], BF16, tag="ps1b")
                nc.tensor.transpose(pst[:], src_blk[:, jc * 128:(jc + 1) * 128],
                                    ident[:])
                nc.any.tensor_copy(
                    out=dst[:, jc, dst_ti * 128:(dst_ti + 1) * 128], in_=pst[:])

        # ---- fused stage A+B: warmup; per ti: load x, LN1, transpose, V ----
        warm = ps_att.tile([128, 128], F32, tag="ps1b", name="warm")
        for _ in range(40):
            nc.tensor.matmul(warm[:], ident[:], ident[:], start=True, stop=True)

        wvp_ctx = ExitStack()
        wvp = wvp_ctx.enter_context(tc.tile_pool(name="wvp", bufs=1))
        wv_sb = wvp.tile([128, NC_, C], FP8, tag="wv")
        for k in range(NC_):
            nc.sync.dma_start(out=wv_sb[:, k, :],
                              in_=wv_d[k * 128:(k + 1) * 128, :])

        wps_ctx = ExitStack()
        wps = wps_ctx.enter_context(tc.tile_pool(name="wps", bufs=1))
        wp_sb = wps.tile([128, NC_, C], FP8, tag="wp")
        for k in range(NC_):
            nc.sync.dma_start(out=wp_sb[:, k, :],
                              in_=wp_d[k * 128:(k + 1) * 128, :])

        with tc.tile_pool(name="mid", bufs=1) as mid:
            vaug = mid.tile([128, NT, H, D + 1], BF16, tag="vaug")
            nc.vector.memset(vaug[:, :, :, D:D + 1], 1.0)

            for ti in range(NT):
                nc.sync.dma_start(out=x_sb[:, ti, :],
                                  in_=x_d[ti * 128:(ti + 1) * 128, :])
                h_blk = layernorm_chunk(x_sb[:, ti, :], ln1_g_rep, ln1_b_rep)
                transpose_into(bufT8, ti, h_blk)

                # V(ti) natural = h @ Wv, fp8 DoubleRow over 4 k-pairs
                ps = ps_big.tile([128, C], F32, tag="psbig", name=f"psv{ti}")
                for jp in range(NC_ // 2):
                    lhsT = bufT8[:, 2 * jp:2 * jp + 2,
                                 ti * 128:(ti + 1) * 128]
                    for off in (0, 512):
                        nc.tensor.matmul(ps[:, off:off + 512], lhsT,
                                         wv_sb[:, 2 * jp:2 * jp + 2,
                                               off:off + 512],
                                         start=(jp == 0), stop=(jp == 3),
                                         perf_mode=DR)
                if bv_rep is not None:
                    vs = scratch.tile([128, C], F32, tag="v_scr")
                    nc.vector.tensor_add(vs[:], ps[:], bv_rep[:])
                    vsrc = vs
                else:
                    vsrc = ps
                nc.vector.tensor_copy(
                    out=vaug[:, ti, :, 0:D],
                    in_=vsrc[:].rearrange("p (h d) -> p h d", d=D))

            # ---- attention, per head-pair c --------------------------------
            recip_dram = dram.tile([16, T], F32)
            with tc.tile_pool(name="qkp", bufs=3) as qkp, \
                 tc.tile_pool(name="wqks", bufs=3) as wqks, \
                 tc.tile_pool(name="epool", bufs=26) as e_pool, \
                 tc.tile_pool(name="dpool", bufs=4) as d_pool, \
                 tc.tile_pool(name="ytsp", bufs=3) as ytsp:
                for c in range(NC_):
                    qk_c = qkp.tile([128, 2, T], BF16, tag="qkc", name=f"qkc_{c}")
                    for sub, m in ((0, c), (1, NC_ + c)):
                        wq = wqks.tile([128, NC_, 128], FP8, tag="wq")
                        nc.sync.dma_start(out=wq[:], in_=wqk_d[m])
                        ps = ps_big.tile([128, T], F32, tag="psbig",
                                         name=f"psqk{m}")
                        for jp in range(NC_ // 2):
                            for off in (0, 512):
                                nc.tensor.matmul(
                                    ps[:, off:off + 512],
                                    wq[:, 2 * jp:2 * jp + 2, :],
                                    bufT8[:, 2 * jp:2 * jp + 2, off:off + 512],
                                    start=(jp == 0), stop=(jp == 3),
                                    perf_mode=DR)
                        if bqk_sb is not None:
                            nc.scalar.activation(out=qk_c[:, sub, :], in_=ps[:],
                                                 func=AF.Identity,
                                                 bias=bqk_sb[:, m:m + 1])
                        else:
                            nc.vector.tensor_copy(out=qk_c[:, sub, :], in_=ps[:])

                    # scores, both heads packed per (j, segment):
                    # strip j covers tq cols [j*128, 1024); segment A =
                    # [j*128, 512) (j<4), segment B = [512, 1024) (all j;
                    # for j>=4 it's [j*128, 1024)). One 1-bank psum each.
                    egrp = {0: {}, 1: {}}   # koff-idx -> {(j, seg): (e, info)}
                    for j in range(NT):
                        segs = []
                        if j < 4:
                            segs.append(("A", j * 128, 512 - j * 128))
                            segs.append(("B", 512, 512))
                        else:
                            segs.append(("B", j * 128, (8 - j) * 128))
                        for seg, col0, ncols in segs:
                            pss = []
                            for hi, koff in ((0, 0), (1, 64)):
                                psc = ps_att.tile(
                                    [128, ncols], F32, tag="ps1b",
                                    name=f"sc_{c}_{hi}_{j}{seg}")
                                nc.tensor.matmul(
                                    psc[:], qk_c[koff:koff + 64, 1,
                                                 j * 128:(j + 1) * 128],
                                    qk_c[koff:koff + 64, 0, col0:col0 + ncols],
                                    start=True, stop=True,
                                    tile_position=(koff, 0))
                                pss.append(psc)
                            for hi, psc in ((0, pss[0]), (1, pss[1])):
                                e = e_pool.tile([128, ncols], BF16, tag="e",
                                                name=f"e_{c}_{hi}_{j}{seg}")
                                nc.scalar.activation(out=e[:], in_=psc[:],
                                                     func=AF.Exp, scale=0.125)
                                # causal mask on the diagonal block
                                doff = j * 128 - col0
                                if 0 <= doff < ncols:
                                    nc.vector.tensor_mul(
                                        e[:, doff:doff + 128],
                                        e[:, doff:doff + 128], mask_sb[:])
                                egrp[hi][(j, seg)] = (e, col0, ncols)

                    for hi, koff in ((0, 0), (1, 64)):
                        h = 2 * c + hi
                        # PV: yT_aug halves [65, 512], accumulate over j
                        ps0 = ps_att.tile([65, 512], F32, tag="ps1b",
                                          name=f"yt0_{h}")
                        ps1 = ps_att.tile([65, 512], F32, tag="ps1b",
                                          name=f"yt1_{h}")
                        for j in range(NT):
                            lhsT = vaug[:, j, h, :]
                            if j < 4:
                                eA, c0A, nA = egrp[hi][(j, "A")]
                                nc.tensor.matmul(
                                    ps0[:, c0A:c0A + nA], lhsT, eA[:],
                                    start=(j == 0), stop=(j == 3))
                                eB, c0B, nB = egrp[hi][(j, "B")]
                                nc.tensor.matmul(
                                    ps1[:, 0:512], lhsT, eB[:],
                                    start=(j == 0), stop=False)
                            else:
                                eB, c0B, nB = egrp[hi][(j, "B")]
                                nc.tensor.matmul(
                                    ps1[:, c0B - 512:c0B - 512 + nB], lhsT,
                                    eB[:], start=False, stop=(j == NT - 1))

                        # softmax denominator: recip on psum row 64, bounce
                        # through DRAM with a step-0 partition AP to broadcast
                        # across 64 partitions, then normalize+cast to fp8
                        # directly off PSUM. yT8 is only read by proj at the
                        # end, so this chain never blocks the PE.
                        den_r = d_pool.tile([128, T], F32, tag="denr",
                                            name=f"denr_{h}")
                        nc.vector.reciprocal(out=den_r[64:65, 0:512],
                                             in_=ps0[64:65, :])
                        nc.vector.reciprocal(out=den_r[64:65, 512:1024],
                                             in_=ps1[64:65, :])
                        nc.sync.dma_start(out=recip_dram[h:h + 1, :],
                                          in_=den_r[64:65, :])
                        rb = d_pool.tile([64, T], F32, tag="rb",
                                         name=f"rb_{h}")
                        rsrc = bass.AP(tensor=recip_dram.tensor,
                                       offset=h * T, ap=[[0, 64], [1, T]])
                        nc.gpsimd.dma_start(out=rb[:], in_=rsrc)
                        yts = ytsp.tile([64, T], FP8, tag="yts",
                                        name=f"yts_{h}")
                        nc.vector.tensor_mul(yts[:, 0:512], ps0[0:64, :],
                                             rb[:, 0:512])
                        nc.vector.tensor_mul(yts[:, 512:1024], ps1[0:64, :],
                                             rb[:, 512:1024])
                        nc.sync.dma_start(
                            out=yT8[koff:koff + 64, c, :], in_=yts[:])

        # ---- proj from yT8, +x -> r1 (in x_sb), fp8 DoubleRow -------------
        for i in range(NT):
            ps = ps_big.tile([128, C], F32, tag="psbig", name=f"pspj{i}")
            for jp in range(NC_ // 2):
                lhsT = yT8[:, 2 * jp:2 * jp + 2, i * 128:(i + 1) * 128]
                for off in (0, 512):
                    nc.tensor.matmul(ps[:, off:off + 512], lhsT,
                                     wp_sb[:, 2 * jp:2 * jp + 2, off:off + 512],
                                     start=(jp == 0), stop=(jp == 3),
                                     perf_mode=DR)
            nc.vector.tensor_add(x_sb[:, i, :], ps[:], x_sb[:, i, :])
            if bproj_rep is not None:
                nc.vector.tensor_add(x_sb[:, i, :], x_sb[:, i, :],
                                     bproj_rep[:])
        wps_ctx.close()
        wvp_ctx.close()

        # x_sb now holds r1. ---- LN2 -> h2T (bufTb, bf16) -------------------
        for ti in range(NT):
            h_blk = layernorm_chunk(x_sb[:, ti, :], ln2_g_rep, ln2_b_rep)
            transpose_into(bufTb, ti, h_blk)

        # ---- fc1 + gelu -> aT (bf16) --------------------------------------
        with tc.tile_pool(name="atp", bufs=1) as atp:
            aT = atp.tile([128, 4 * NC_, T], BF16, tag="aT")
            with tc.tile_pool(name="wfcs", bufs=3) as wfcs:
                for m in range(4 * NC_):
                    wf = wfcs.tile([128, NC_, 128], BF16, tag="wf")
                    nc.sync.dma_start(out=wf[:], in_=wfc_d[m])
                    ps = ps_big.tile([128, T], F32, tag="psbig",
                                     name=f"psf1_{m}")
                    for k in range(NC_):
                        for off in (0, 512):
                            nc.tensor.matmul(ps[:, off:off + 512], wf[:, k, :],
                                             bufTb[:, k, off:off + 512],
                                             start=(k == 0), stop=(k == NC_ - 1))
                    bias = bfc_sb[:, m:m + 1] if bfc_sb is not None else 0.0
                    act_fn = AF.Tanh if _SIM_GELU_TANH else AF.Gelu_apprx_tanh
                    nc.scalar.activation(out=aT[:, m, :], in_=ps[:],
                                         func=act_fn, bias=bias)

            # ---- fc2 + residual -> out (bf16) -----------------------------
            ps_ctx.close()
            with tc.tile_pool(name="ps_fc2", bufs=4, space="PSUM") as ps_fc2, \
                 tc.tile_pool(name="wmlps", bufs=3) as wmlps:
                for half in range(2):
                    iis = list(range(half * 4, half * 4 + 4))
                    psums = {}
                    for i in iis:
                        psums[i] = ps_fc2.tile([128, C], F32, tag="psf2",
                                               name=f"psf2_{i}")
                    for k in range(4 * NC_):
                        wm = wmlps.tile([128, C], BF16, tag="wm")
                        nc.sync.dma_start(out=wm[:],
                                          in_=wmlp_d[k * 128:(k + 1) * 128, :])
                        for i in iis:
                            lhsT = aT[:, k, i * 128:(i + 1) * 128]
                            for off in (0, 512):
                                nc.tensor.matmul(
                                    psums[i][:, off:off + 512], lhsT,
                                    wm[:, off:off + 512],
                                    start=(k == 0), stop=(k == 4 * NC_ - 1))
                    for i in iis:
                        o = o_pool.tile([128, C], F32, tag="o")
                        nc.vector.tensor_add(o[:], psums[i][:], x_sb[:, i, :])
                        if bmlp_rep is not None:
                            nc.vector.tensor_add(o[:], o[:], bmlp_rep[:])
                        nc.sync.dma_start(out=out_d[i * 128:(i + 1) * 128, :],
                                          in_=o[:])


# ---------------------------------------------------------------------------
_CACHE = {}


def _prearrange_kxm(w, nm, np_dt):
    """[C, nm*128] -> [nm, 128, C//128, 128] so chunk DMAs are contiguous.

    out[m, p, ko, mm] = w[ko*128 + p, m*128 + mm]
    """
    cin = w.shape[0]
    a = w.reshape(cin // 128, 128, nm, 128)        # [ko, p, m, mm]
    a = np.transpose(a, (2, 1, 0, 3))              # [m, p, ko, mm]
    return np.ascontiguousarray(a.astype(np_dt))


def _build_in_maps(inputs):
    x = np.asarray(inputs["x"], dtype=np.float32)
    w_qkv = np.asarray(inputs["w_qkv"], dtype=np.float32)
    b_qkv = np.asarray(inputs["b_qkv"], dtype=np.float32)

    flags = {
        "b_qk": bool(np.any(b_qkv[:2 * C])),
        "b_v": bool(np.any(b_qkv[2 * C:])),
        "b_proj": bool(np.any(inputs["b_attn_proj"])),
        "b_fc": bool(np.any(inputs["b_fc"])),
        "b_mlp": bool(np.any(inputs["b_mlp_proj"])),
        "ln1_g": not bool(np.allclose(np.asarray(inputs["ln1_g"]), 1.0)),
        "ln1_b": bool(np.any(inputs["ln1_b"])),
        "ln2_g": not bool(np.allclose(np.asarray(inputs["ln2_g"]), 1.0)),
        "ln2_b": bool(np.any(inputs["ln2_b"])),
    }

    bf = ml_dtypes.bfloat16
    f8 = ml_dtypes.float8_e4m3
    shared = {
        "w_qk": _prearrange_kxm(w_qkv[:, :2 * C], 2 * NC_, f8),
        "w_fc": _prearrange_kxm(np.asarray(inputs["w_fc"], np.float32),
                                4 * NC_, bf),
        "w_v": np.ascontiguousarray(w_qkv[:, 2 * C:]).astype(f8),
        "w_proj": np.asarray(inputs["w_attn_proj"], np.float32).astype(f8),
        "w_mlp": np.asarray(inputs["w_mlp_proj"], np.float32).astype(bf),
        "mask_ut": np.triu(np.ones((128, 128))).astype(bf),
    }
    if flags["b_qk"]:
        shared["b_qk"] = np.ascontiguousarray(b_qkv[:2 * C].reshape(2 * NC_, 128).T)
    if flags["b_v"]:
        shared["b_v"] = np.ascontiguousarray(b_qkv[2 * C:])
    if flags["b_proj"]:
        shared["b_proj"] = np.asarray(inputs["b_attn_proj"], np.float32)
    if flags["b_fc"]:
        shared["b_fc"] = np.ascontiguousarray(
            np.asarray(inputs["b_fc"], np.float32).reshape(4 * NC_, 128).T)
    if flags["b_mlp"]:
        shared["b_mlp"] = np.asarray(inputs["b_mlp_proj"], np.float32)
    for nm in ("ln1_g", "ln1_b", "ln2_g", "ln2_b"):
        if flags[nm]:
            shared[nm] = np.asarray(inputs[nm], np.float32)

    in_maps = [dict(shared, x=np.ascontiguousarray(x[c])) for c in range(x.shape[0])]
    return flags, in_maps


def kernel_run(inputs, trace=False, trace_kwargs=None):
    """Build (cached), run on 8 cores, return (full_output, BassKernelResults)."""
    from concourse.bass_utils import run_bass_kernel_spmd

    flags, in_maps = _build_in_maps(inputs)
    key = tuple(sorted(flags.items()))
    if key not in _CACHE:
        _CACHE[key] = build_nc(flags)
    nc = _CACHE[key]
    res = run_bass_kernel_spmd(nc, in_maps, core_ids=list(range(8)),
                               trace=trace, trace_kwargs=trace_kwargs or {})
    out = np.stack([res.results[c]["out"] for c in range(8)]).astype(np.float32)
    return out, res


def kernel(**inputs) -> np.ndarray:
    out, _ = kernel_run(inputs, trace=False)
    return out


# revision 16
# speedup vs baseline: 1.2125x; 1.2125x over previous
"""Trainium2 Bass kernel for a dense transformer block (B=8, T=1024, C=1024, H=16).

Data-parallel over batch across the 8 NeuronCores (one batch element per core,
weights replicated, no collectives).

Per-core dataflow v2 (fp8 DoubleRow for qkv/proj, bf16 MLP, packed scores):

  per ti: x chunk DMA -> LN1 -> PE transpose -> h1T fp8 [C,T]
          -> V(ti) = h @ Wv   (fp8 DoubleRow)        -> vaug bf16 [T,H,D+1]
  per pair c (heads 2c, 2c+1):
    qkT chunks (fp8 DoubleRow, lhsT=Wqk)             -> qk_c bf16 [128, 2, T]
    scores S^T: both heads packed on PE row halves via tile_position
    E = exp(S/8) on ACT (bf16), causal diag mask post-exp (DVE)
    yT_aug halves = E^T-blocks @ [v|1]  (bf16)       -> psum [65, 512] x2
    den recip on psum row 64 (DVE) -> partition_broadcast (GPSIMD)
    yT = psum * recip -> fp8, DMA into yT8 (handles odd-head partition shift)
  proj = y @ Wp (fp8 DoubleRow) + x                  -> r1 (natural f32)
  LN2 -> h2T bf16; fc1+gelu (bf16) -> aT; fc2 + residual (bf16) -> out

All PSUM accumulation fp32; LN statistics and residual stream fp32.
"""
import sys

sys.path.insert(0, "/opt/trn_rl_repo")

import numpy as np
import ml_dtypes

import concourse.bass as bass
import concourse.tile as tile
from concourse import mybir
from concourse.masks import make_identity
from concourse.vector_clock import ScopedClock

F32 = mybir.dt.float32
BF16 = mybir.dt.bfloat16
FP8 = mybir.dt.float8e4
AF = mybir.ActivationFunctionType
DR = mybir.MatmulPerfMode.DoubleRow

T, C, H, D = 1024, 1024, 16, 64
NT = T // 128   # 8 token chunks
NC_ = C // 128  # 8 feature chunks
EPS = 1e-5

# ---------------------------------------------------------------------------
# Walrus in this container rejects >1 sem-wait per CTRL instruction; split the
# Tile tail-drain's waits across nop carriers. The carrier nops (wait-only, no
# updates) break CoreSim's race detector, so sim builds disable this shim.
_MAX_WAITS = 1
_WALRUS_COMPAT = True
_SIM_GELU_TANH = False

_orig_drain_and_barrier = tile.TileContext._drain_and_barrier


def _patched_drain_and_barrier(self, tick_clock, wait_clock):
    if not _WALRUS_COMPAT:
        return _orig_drain_and_barrier(self, tick_clock, wait_clock)
    nc = self.nc
    carrier = nc.sync.nop(nofuse=True)
    wait_clock.add_sem_waits(carrier.ins, ScopedClock({None: tick_clock.global_clock}))
    si = carrier.ins.sync_info
    waits = list(si.on_wait) if si and si.on_wait else []
    if len(waits) > _MAX_WAITS:
        si.on_wait = waits[:_MAX_WAITS]
        for k in range(_MAX_WAITS, len(waits), _MAX_WAITS):
            extra = nc.sync.nop(nofuse=True)
            esi = extra.ins.sync_info
            if esi is None:
                extra.ins.sync_info = mybir.SyncInfo(
                    on_wait=waits[k:k + _MAX_WAITS], on_update=[]
                )
            else:
                esi.on_wait = waits[k:k + _MAX_WAITS]
    nc.sync.drain()
    nc.all_engine_barrier()
    popped = nc._tile_sem_poison_stack.pop()
    assert popped is self._sem_poison
    nc.clear_and_free_semaphores(list(self.sems.allocated().values()))
    nc.all_engine_barrier()


tile.TileContext._drain_and_barrier = _patched_drain_and_barrier


def _split_sync_waits(nc, max_waits=1):
    """Walrus here rejects >1 sem-wait per instruction; hoist extras onto
    preceding same-engine nops."""
    ctr = 0
    for f in nc.m.functions:
        for b in f.blocks:
            out = []
            for ins in b.instructions:
                si = ins.sync_info
                ws = list(si.on_wait) if si and si.on_wait else []
                if len(ws) > max_waits:
                    extra, keep = ws[:-max_waits], ws[-max_waits:]
                    for i in range(0, len(extra), max_waits):
                        nop = mybir.InstNoOp(
                            name=f"wsplit-{ctr}", engine=ins.engine,
                            sync_info=mybir.SyncInfo(
                                on_wait=extra[i:i + max_waits], on_update=[]))
                        ctr += 1
                        out.append(nop)
                    si.on_wait = keep
                out.append(ins)
            b.instructions = out


def build_nc(flags):
    nc = bass.Bass()

    x_d = nc.dram_tensor("x", [T, C], F32, kind="ExternalInput")
    # host-prearranged: [m_chunk, p, ko, 128] so per-chunk DMAs are contiguous
    wqk_d = nc.dram_tensor("w_qk", [2 * NC_, 128, NC_, 128], FP8,
                           kind="ExternalInput")
    wfc_d = nc.dram_tensor("w_fc", [4 * NC_, 128, NC_, 128], BF16,
                           kind="ExternalInput")
    wv_d = nc.dram_tensor("w_v", [C, C], FP8, kind="ExternalInput")
    wp_d = nc.dram_tensor("w_proj", [C, C], FP8, kind="ExternalInput")
    wmlp_d = nc.dram_tensor("w_mlp", [4 * C, C], BF16, kind="ExternalInput")
    mask_d = nc.dram_tensor("mask_ut", [128, 128], BF16, kind="ExternalInput")
    opt = {}
    if flags["b_qk"]:
        opt["b_qk"] = nc.dram_tensor("b_qk", [128, 2 * NC_], F32, kind="ExternalInput")
    if flags["b_v"]:
        opt["b_v"] = nc.dram_tensor("b_v", [C], F32, kind="ExternalInput")
    if flags["b_proj"]:
        opt["b_proj"] = nc.dram_tensor("b_proj", [C], F32, kind="ExternalInput")
    if flags["b_fc"]:
        opt["b_fc"] = nc.dram_tensor("b_fc", [128, 4 * NC_], F32, kind="ExternalInput")
    if flags["b_mlp"]:
        opt["b_mlp"] = nc.dram_tensor("b_mlp", [C], F32, kind="ExternalInput")
    for nm in ("ln1_g", "ln1_b", "ln2_g", "ln2_b"):
        if flags[nm]:
            opt[nm] = nc.dram_tensor(nm, [C], F32, kind="ExternalInput")
    out_d = nc.dram_tensor("out", [T, C], F32, kind="ExternalOutput")

    with tile.TileContext(nc) as tc:
        _build_body(nc, tc, flags, x_d, wqk_d, wv_d, wp_d, wfc_d, wmlp_d,
                    mask_d, opt, out_d)
    if _WALRUS_COMPAT:
        _split_sync_waits(nc)
    return nc


def _build_body(nc, tc, flags, x_d, wqk_d, wv_d, wp_d, wfc_d, wmlp_d,
                mask_d, opt, out_d):
    from contextlib import ExitStack

    ctx = ExitStack()
    with ctx:
        const = ctx.enter_context(tc.tile_pool(name="const", bufs=1))
        big = ctx.enter_context(tc.tile_pool(name="big", bufs=1))
        scratch = ctx.enter_context(tc.tile_pool(name="scratch", bufs=3))
        small = ctx.enter_context(tc.tile_pool(name="small", bufs=8))
        o_pool = ctx.enter_context(tc.tile_pool(name="opool", bufs=2))
        dram = ctx.enter_context(tc.tile_pool(name="dram", bufs=1, space="DRAM"))

        # ---- constants -----------------------------------------------------
        ident = const.tile([128, 128], BF16, tag="ident")
        make_identity(nc, ident)
        mask_sb = const.tile([128, 128], BF16, tag="mask")
        nc.sync.dma_start(mask_sb[:], mask_d[:])
        eps_t = const.tile([128, 1], F32, tag="eps")
        nc.vector.memset(eps_t[:], EPS)

        def rep128(vec_dram):
            t = const.tile([128, C], F32, tag=f"rep_{vec_dram.tensor.name}")
            src = bass.AP(tensor=vec_dram.tensor, offset=0, ap=[[0, 128], [1, C]])
            nc.gpsimd.dma_start(out=t[:], in_=src)
            return t

        ln1_g_rep = rep128(opt["ln1_g"]) if flags["ln1_g"] else None
        ln1_b_rep = rep128(opt["ln1_b"]) if flags["ln1_b"] else None
        ln2_g_rep = rep128(opt["ln2_g"]) if flags["ln2_g"] else None
        ln2_b_rep = rep128(opt["ln2_b"]) if flags["ln2_b"] else None
        bv_rep = rep128(opt["b_v"]) if flags["b_v"] else None
        bproj_rep = rep128(opt["b_proj"]) if flags["b_proj"] else None
        bmlp_rep = rep128(opt["b_mlp"]) if flags["b_mlp"] else None
        bqk_sb = None
        if flags["b_qk"]:
            bqk_sb = const.tile([128, 2 * NC_], F32, tag="bqk")
            nc.sync.dma_start(bqk_sb[:], opt["b_qk"][:])
        bfc_sb = None
        if flags["b_fc"]:
            bfc_sb = const.tile([128, 4 * NC_], F32, tag="bfc")
            nc.sync.dma_start(bfc_sb[:], opt["b_fc"][:])

        # ---- persistent tiles ---------------------------------------------
        x_sb = big.tile([128, NT, C], F32, tag="x")        # x, then r1 in place
        bufT8 = big.tile([128, NC_, T], FP8, tag="bufT8")  # h1T fp8
        bufTb = big.tile([128, NC_, T], BF16, tag="bufTb")  # h2T bf16
        yT8 = big.tile([128, NC_, T], FP8, tag="yT8")      # normalized y^T fp8

        def layernorm_chunk(src_slice, g_rep, b_rep):
            stats = small.tile([128, 2, 6], F32, tag="bn_stats")
            xr = src_slice.rearrange("p (s f) -> p s f", f=512)
            for s in range(2):
                nc.vector.bn_stats(out=stats[:, s, :], in_=xr[:, s, :])
            mv = small.tile([128, 2], F32, tag="bn_mv")
            nc.vector.bn_aggr(out=mv[:], in_=stats[:])
            rstd = small.tile([128, 1], F32, tag="rstd")
            nc.scalar.activation(out=rstd[:], in_=mv[:, 1:2], func=AF.Sqrt,
                                 bias=eps_t[:], scale=1.0)
            nc.vector.reciprocal(out=rstd[:], in_=rstd[:])
            h_blk = scratch.tile([128, C], BF16, tag="h_blk")
            nc.vector.tensor_scalar(
                out=h_blk[:], in0=src_slice, scalar1=mv[:, 0:1], scalar2=rstd[:],
                op0=mybir.AluOpType.subtract, op1=mybir.AluOpType.mult)
            if g_rep is not None:
                nc.vector.tensor_mul(h_blk[:], h_blk[:], g_rep[:])
            if b_rep is not None:
                nc.vector.tensor_add(h_blk[:], h_blk[:], b_rep[:])
            return h_blk

        # PSUM pool: four 2-bank slots ([128, 1024] f32) shared by all big
        # accumulators, score groups, PV tiles, transposes and warmup.
        ps_ctx = ExitStack()
        ps2b = ps_ctx.enter_context(
            tc.tile_pool(name="ps2b", bufs=4, space="PSUM"))

        def transpose_into(dst, dst_ti, src_blk):
            for jc in range(NC_):
                pst = ps2b.tile([128, 128], BF16, tag="ps2b")
                nc.tensor.transpose(pst[:], src_blk[:, jc * 128:(jc + 1) * 128],
                                    ident[:])
                nc.any.tensor_copy(
                    out=dst[:, jc, dst_ti * 128:(dst_ti + 1) * 128], in_=pst[:])

        # ---- fused stage A+B: warmup; per ti: load x, LN1, transpose, V ----
        # x ti0 first so the LN->transpose chain starts ASAP; wv needed at
        # V(ti0) ~10us in; wp deferred (proj reads it much later).
        nc.sync.dma_start(out=x_sb[:, 0, :], in_=x_d[0:128, :])

        warm = ps2b.tile([128, 128], F32, tag="ps2b", name="warm")
        for _ in range(32):
            nc.tensor.matmul(warm[:], ident[:], ident[:], start=True, stop=True)

        wvp_ctx = ExitStack()
        wvp = wvp_ctx.enter_context(tc.tile_pool(name="wvp", bufs=1))
        wv_sb = wvp.tile([128, NC_, C], FP8, tag="wv")
        for k in range(NC_):
            nc.sync.dma_start(out=wv_sb[:, k, :],
                              in_=wv_d[k * 128:(k + 1) * 128, :])

        wps_ctx = ExitStack()
        wps = wps_ctx.enter_context(tc.tile_pool(name="wps", bufs=1))
        wp_sb = wps.tile([128, NC_, C], FP8, tag="wp")

        with tc.tile_pool(name="mid", bufs=1) as mid:
            vaug = mid.tile([128, NT, H, D + 1], BF16, tag="vaug")
            nc.vector.memset(vaug[:, :, :, D:D + 1], 1.0)

            for ti in range(NT):
                if ti > 0:
                    nc.sync.dma_start(out=x_sb[:, ti, :],
                                      in_=x_d[ti * 128:(ti + 1) * 128, :])
                h_blk = layernorm_chunk(x_sb[:, ti, :], ln1_g_rep, ln1_b_rep)
                transpose_into(bufT8, ti, h_blk)

                # V(ti) natural = h @ Wv, fp8 DoubleRow over 4 k-pairs
                ps = ps2b.tile([128, C], F32, tag="ps2b", name=f"psv{ti}")
                for jp in range(NC_ // 2):
                    lhsT = bufT8[:, 2 * jp:2 * jp + 2,
                                 ti * 128:(ti + 1) * 128]
                    for off in (0, 512):
                        nc.tensor.matmul(ps[:, off:off + 512], lhsT,
                                         wv_sb[:, 2 * jp:2 * jp + 2,
                                               off:off + 512],
                                         start=(jp == 0), stop=(jp == 3),
                                         perf_mode=DR)
                if bv_rep is not None:
                    vs = scratch.tile([128, C], F32, tag="v_scr")
                    nc.vector.tensor_add(vs[:], ps[:], bv_rep[:])
                    vsrc = vs
                else:
                    vsrc = ps
                nc.vector.tensor_copy(
                    out=vaug[:, ti, :, 0:D],
                    in_=vsrc[:].rearrange("p (h d) -> p h d", d=D))

            for k in range(NC_):
                nc.sync.dma_start(out=wp_sb[:, k, :],
                                  in_=wp_d[k * 128:(k + 1) * 128, :])

            # ---- attention, per head-pair c --------------------------------
            den_dram = dram.tile([16, T], F32)
            recip_dram = dram.tile([16, T], F32)
            with tc.tile_pool(name="qkp", bufs=3) as qkp, \
                 tc.tile_pool(name="wqks", bufs=3) as wqks, \
                 tc.tile_pool(name="epool", bufs=14) as e_pool, \
                 tc.tile_pool(name="scrp", bufs=3) as scrp, \
                 tc.tile_pool(name="dpool", bufs=3) as d_pool, \
                 tc.tile_pool(name="rbp", bufs=3) as rbp, \
                 tc.tile_pool(name="ytsp", bufs=3) as ytsp:
                # score col groups: j-strips packed so each psum tile is
                # [128, <=1024] (<=2 banks); 6 exp calls per head.
                _GRPS = ((0,), (1,), (2,), (3,), (4, 5), (6, 7))
                for c in range(NC_):
                    qk_c = qkp.tile([128, 2, T], BF16, tag="qkc", name=f"qkc_{c}")
                    for sub, m in ((0, c), (1, NC_ + c)):
                        wq = wqks.tile([128, NC_, 128], FP8, tag="wq")
                        nc.sync.dma_start(out=wq[:], in_=wqk_d[m])
                        ps = ps2b.tile([128, T], F32, tag="ps2b",
                                       name=f"psqk{m}")
                        for jp in range(NC_ // 2):
                            for off in (0, 512):
                                nc.tensor.matmul(
                                    ps[:, off:off + 512],
                                    wq[:, 2 * jp:2 * jp + 2, :],
                                    bufT8[:, 2 * jp:2 * jp + 2, off:off + 512],
                                    start=(jp == 0), stop=(jp == 3),
                                    perf_mode=DR)
                        if bqk_sb is not None:
                            nc.scalar.activation(out=qk_c[:, sub, :], in_=ps[:],
                                                 func=AF.Identity,
                                                 bias=bqk_sb[:, m:m + 1])
                        else:
                            nc.vector.tensor_copy(out=qk_c[:, sub, :], in_=ps[:])

                    # scores: per col group, both heads packed on PE row
                    # halves via tile_position; exp once per (group, head).
                    egrp = {0: {}, 1: {}}   # hi -> {j: (e_tile, col_off)}
                    for grp in _GRPS:
                        w_g = sum((8 - j) * 128 for j in grp)
                        pss = {}
                        for hi, koff in ((0, 0), (1, 64)):
                            psc = ps2b.tile([128, w_g], F32, tag="ps2b",
                                            name=f"sc_{c}_{hi}_{grp[0]}")
                            col = 0
                            for j in grp:
                                rem = (8 - j) * 128
                                lhsT = qk_c[koff:koff + 64, 1,
                                            j * 128:(j + 1) * 128]
                                off = col
                                src_off = j * 128
                                while off < col + rem:
                                    n = min(col + rem - off,
                                            512 - (off % 512))
                                    nc.tensor.matmul(
                                        psc[:, off:off + n], lhsT,
                                        qk_c[koff:koff + 64, 0,
                                             src_off:src_off + n],
                                        start=True, stop=True,
                                        tile_position=(koff, 0))
                                    off += n
                                    src_off += n
                                col += rem
                            pss[hi] = psc
                        for hi in (0, 1):
                            e = e_pool.tile([128, w_g], BF16, tag="e",
                                            name=f"e_{c}_{hi}_{grp[0]}")
                            nc.scalar.activation(out=e[:], in_=pss[hi][:],
                                                 func=AF.Exp, scale=0.125)
                            col = 0
                            for j in grp:
                                nc.vector.tensor_mul(
                                    e[:, col:col + 128], e[:, col:col + 128],
                                    mask_sb[:])
                                egrp[hi][j] = (e, col)
                                col += (8 - j) * 128

                    scrs = {}
                    for hi, koff in ((0, 0), (1, 64)):
                        h = 2 * c + hi
                        # PV: one [65, 1024] 2-bank tile; MMs hit bank-aligned
                        # 512-col halves, accumulating over j.
                        ps = ps2b.tile([65, T], F32, tag="ps2b",
                                       name=f"yt_{h}")
                        for j in range(NT):
                            lhsT = vaug[:, j, h, :]
                            et, eo = egrp[hi][j]
                            if j <= 3:
                                nA = (4 - j) * 128
                                nc.tensor.matmul(
                                    ps[:, j * 128:512], lhsT,
                                    et[:, eo:eo + nA],
                                    start=(j == 0), stop=(j == 3))
                                nc.tensor.matmul(
                                    ps[:, 512:1024], lhsT,
                                    et[:, eo + nA:eo + nA + 512],
                                    start=(j == 0), stop=False)
                            else:
                                nB = (8 - j) * 128
                                nc.tensor.matmul(
                                    ps[:, j * 128:j * 128 + nB], lhsT,
                                    et[:, eo:eo + nB],
                                    start=False, stop=(j == NT - 1))

                        # evict y rows (f32) + ship den row to DRAM; psum
                        # frees immediately, normalization happens off the
                        # critical path (yT8 only read by proj at the end).
                        scr = scrp.tile([65, T], F32, tag="scr",
                                        name=f"scr_{h}")
                        nc.vector.tensor_copy(out=scr[:], in_=ps[:])
                        nc.sync.dma_start(out=den_dram[h:h + 1, :],
                                          in_=scr[64:65, :])
                        scrs[hi] = scr

                    # batched denominators for the pair: regather the 2048
                    # den values as [128, 16] so all DVE lanes work on the
                    # reciprocal (~0.1us), then scatter back flat.
                    den2 = d_pool.tile([128, 16], F32, tag="den2",
                                       name=f"den2_{c}")
                    dsrc = bass.AP(tensor=den_dram.tensor, offset=2 * c * T,
                                   ap=[[16, 128], [1, 16]])
                    nc.sync.dma_start(out=den2[:], in_=dsrc)
                    rec2 = d_pool.tile([128, 16], F32, tag="rec2",
                                       name=f"rec2_{c}")
                    nc.vector.reciprocal(out=rec2[:], in_=den2[:])
                    rdst = bass.AP(tensor=recip_dram.tensor,
                                   offset=2 * c * T, ap=[[16, 128], [1, 16]])
                    nc.sync.dma_start(out=rdst, in_=rec2[:])
                    for hi, koff in ((0, 0), (1, 64)):
                        h = 2 * c + hi
                        rb = rbp.tile([64, T], F32, tag="rb", name=f"rb_{h}")
                        rsrc = bass.AP(tensor=recip_dram.tensor,
                                       offset=h * T, ap=[[0, 64], [1, T]])
                        nc.sync.dma_start(out=rb[:], in_=rsrc)
                        yts = ytsp.tile([64, T], FP8, tag="yts",
                                        name=f"yts_{h}")
                        nc.vector.tensor_mul(yts[:], scrs[hi][0:64, :], rb[:])
                        nc.sync.dma_start(
                            out=yT8[koff:koff + 64, c, :], in_=yts[:])

        # ---- proj from yT8, +x -> r1 (in x_sb), fp8 DoubleRow -------------
        for i in range(NT):
            ps = ps2b.tile([128, C], F32, tag="ps2b", name=f"pspj{i}")
            for jp in range(NC_ // 2):
                lhsT = yT8[:, 2 * jp:2 * jp + 2, i * 128:(i + 1) * 128]
                for off in (0, 512):
                    nc.tensor.matmul(ps[:, off:off + 512], lhsT,
                                     wp_sb[:, 2 * jp:2 * jp + 2, off:off + 512],
                                     start=(jp == 0), stop=(jp == 3),
                                     perf_mode=DR)
            nc.vector.tensor_add(x_sb[:, i, :], ps[:], x_sb[:, i, :])
            if bproj_rep is not None:
                nc.vector.tensor_add(x_sb[:, i, :], x_sb[:, i, :],
                                     bproj_rep[:])
        wps_ctx.close()
        wvp_ctx.close()

        # x_sb now holds r1. ---- LN2 -> h2T (bufTb, bf16) -------------------
        for ti in range(NT):
            h_blk = layernorm_chunk(x_sb[:, ti, :], ln2_g_rep, ln2_b_rep)
            transpose_into(bufTb, ti, h_blk)

        # ---- fc1 + gelu -> aT (bf16) --------------------------------------
        with tc.tile_pool(name="atp", bufs=1) as atp:
            aT = atp.tile([128, 4 * NC_, T], BF16, tag="aT")
            with tc.tile_pool(name="wfcs", bufs=3) as wfcs:
                for m in range(4 * NC_):
                    wf = wfcs.tile([128, NC_, 128], BF16, tag="wf")
                    nc.sync.dma_start(out=wf[:], in_=wfc_d[m])
                    ps = ps2b.tile([128, T], F32, tag="ps2b",
                                     name=f"psf1_{m}")
                    for k in range(NC_):
                        for off in (0, 512):
                            nc.tensor.matmul(ps[:, off:off + 512], wf[:, k, :],
                                             bufTb[:, k, off:off + 512],
                                             start=(k == 0), stop=(k == NC_ - 1))
                    bias = bfc_sb[:, m:m + 1] if bfc_sb is not None else 0.0
                    act_fn = AF.Tanh if _SIM_GELU_TANH else AF.Gelu_apprx_tanh
                    nc.scalar.activation(out=aT[:, m, :], in_=ps[:],
                                         func=act_fn, bias=bias)

            # ---- fc2 + residual -> out (bf16) -----------------------------
            ps_ctx.close()
            with tc.tile_pool(name="ps_fc2", bufs=4, space="PSUM") as ps_fc2, \
                 tc.tile_pool(name="wmlps", bufs=3) as wmlps:
                for half in range(2):
                    iis = list(range(half * 4, half * 4 + 4))
                    psums = {}
                    for i in iis:
                        psums[i] = ps_fc2.tile([128, C], F32, tag="psf2",
                                               name=f"psf2_{i}")
                    for k in range(4 * NC_):
                        wm = wmlps.tile([128, C], BF16, tag="wm")
                        nc.sync.dma_start(out=wm[:],
                                          in_=wmlp_d[k * 128:(k + 1) * 128, :])
                        for i in iis:
                            lhsT = aT[:, k, i * 128:(i + 1) * 128]
                            for off in (0, 512):
                                nc.tensor.matmul(
                                    psums[i][:, off:off + 512], lhsT,
                                    wm[:, off:off + 512],
                                    start=(k == 0), stop=(k == 4 * NC_ - 1))
                    for i in iis:
                        o = o_pool.tile([128, C], F32, tag="o")
                        nc.vector.tensor_add(o[:], psums[i][:], x_sb[:, i, :])
                        if bmlp_rep is not None:
                            nc.vector.tensor_add(o[:], o[:], bmlp_rep[:])
                        nc.sync.dma_start(out=out_d[i * 128:(i + 1) * 128, :],
                                          in_=o[:])


# ---------------------------------------------------------------------------
_CACHE = {}


def _prearrange_kxm(w, nm, np_dt):
    """[C, nm*128] -> [nm, 128, C//128, 128] so chunk DMAs are contiguous.

    out[m, p, ko, mm] = w[ko*128 + p, m*128 + mm]
    """
    cin = w.shape[0]
    a = w.reshape(cin // 128, 128, nm, 128)        # [ko, p, m, mm]
    a = np.transpose(a, (2, 1, 0, 3))              # [m, p, ko, mm]
    return np.ascontiguousarray(a.astype(np_dt))


def _build_in_maps(inputs):
    x = np.asarray(inputs["x"], dtype=np.float32)
    w_qkv = np.asarray(inputs["w_qkv"], dtype=np.float32)
    b_qkv = np.asarray(inputs["b_qkv"], dtype=np.float32)

    flags = {
        "b_qk": bool(np.any(b_qkv[:2 * C])),
        "b_v": bool(np.any(b_qkv[2 * C:])),
        "b_proj": bool(np.any(inputs["b_attn_proj"])),
        "b_fc": bool(np.any(inputs["b_fc"])),
        "b_mlp": bool(np.any(inputs["b_mlp_proj"])),
        "ln1_g": not bool(np.allclose(np.asarray(inputs["ln1_g"]), 1.0)),
        "ln1_b": bool(np.any(inputs["ln1_b"])),
        "ln2_g": not bool(np.allclose(np.asarray(inputs["ln2_g"]), 1.0)),
        "ln2_b": bool(np.any(inputs["ln2_b"])),
    }

    bf = ml_dtypes.bfloat16
    f8 = ml_dtypes.float8_e4m3
    shared = {
        "w_qk": _prearrange_kxm(w_qkv[:, :2 * C], 2 * NC_, f8),
        "w_fc": _prearrange_kxm(np.asarray(inputs["w_fc"], np.float32),
                                4 * NC_, bf),
        "w_v": np.ascontiguousarray(w_qkv[:, 2 * C:]).astype(f8),
        "w_proj": np.asarray(inputs["w_attn_proj"], np.float32).astype(f8),
        "w_mlp": np.asarray(inputs["w_mlp_proj"], np.float32).astype(bf),
        "mask_ut": np.triu(np.ones((128, 128))).astype(bf),
    }
    if flags["b_qk"]:
        shared["b_qk"] = np.ascontiguousarray(b_qkv[:2 * C].reshape(2 * NC_, 128).T)
    if flags["b_v"]:
        shared["b_v"] = np.ascontiguousarray(b_qkv[2 * C:])
    if flags["b_proj"]:
        shared["b_proj"] = np.asarray(inputs["b_attn_proj"], np.float32)
    if flags["b_fc"]:
        shared["b_fc"] = np.ascontiguousarray(
            np.asarray(inputs["b_fc"], np.float32).reshape(4 * NC_, 128).T)
    if flags["b_mlp"]:
        shared["b_mlp"] = np.asarray(inputs["b_mlp_proj"], np.float32)
    for nm in ("ln1_g", "ln1_b", "ln2_g", "ln2_b"):
        if flags[nm]:
            shared[nm] = np.asarray(inputs[nm], np.float32)

    in_maps = [dict(shared, x=np.ascontiguousarray(x[c])) for c in range(x.shape[0])]
    return flags, in_maps


def kernel_run(inputs, trace=False, trace_kwargs=None):
    """Build (cached), run on 8 cores, return (full_output, BassKernelResults)."""
    from concourse.bass_utils import run_bass_kernel_spmd

    flags, in_maps = _build_in_maps(inputs)
    key = tuple(sorted(flags.items()))
    if key not in _CACHE:
        _CACHE[key] = build_nc(flags)
    nc = _CACHE[key]
    res = run_bass_kernel_spmd(nc, in_maps, core_ids=list(range(8)),
                               trace=trace, trace_kwargs=trace_kwargs or {})
    out = np.stack([res.results[c]["out"] for c in range(8)]).astype(np.float32)
    return out, res


def kernel(**inputs) -> np.ndarray:
    out, _ = kernel_run(inputs, trace=False)
    return out
